# revision 10
# baseline (speedup 1.0000x reference)
"""Fused multi-head causal self-attention block for Trainium2 (Bass/Tile).

Problem: y = MHA(x; Wq,bq,Wk,bk,Wv,bv,Wo,bo) with
  B=512, N=128 tokens, C=512 channels, H=8 heads, D=64, causal mask applied
  before the 1/sqrt(D) scaling (mask * -1e5 -> exp underflows to exactly 0).

Sharding: data-parallel over batch across 8 NeuronCores (64 batch elems per
core), weights replicated, no collectives.

Key structure (v2 — fp8 DoubleRow projections):
  - x is shipped from the host PRE-TRANSPOSED and pre-quantized to fp8-e4m3
    as a hi+lo residual pair (xT ~= xh + xl, elementwise e4m3 split), laid
    out group-contiguous [ng, 128, CT, G*N]. No on-device cast/transpose.
  - All four projections run as fp8 DoubleRow matmuls (2 K-tiles of 128 per
    instruction, 0.5 PE cycles/row):
      Q,K: psum = xh@W8 + xl@W8            (W8 = e4m3(W), plain)
      V:   psum = xh@Wh + xh@Wl + xl@Wh    (Wh/Wl = e4m3 hi/lo of 16*Wv;
           the x16 pre-scale keeps Wl out of e4m3's subnormal floor; the
           1/16 rescale rides the psum->SBUF copy for free)
    Numerics (vs fp32 reference, measured): rel-to-max ~0.0096 — the V/ao
    path is error-sensitive (linear into y) so V gets the 3-term residual;
    Q/K errors are softmax-dampened so 2-term suffices.
  - Bias algebra: softmax(s + row-const) = softmax(s) row-wise, so
    Q.bk-cross terms drop => K carries NO bias; Q keeps bq (added on the
    psum->SBUF copy). bv folds into bo (attn rows sum to 1), and bo itself
    is added on the HOST after the gather (y_dev is bias-free).
  - scores TRANSPOSED [k,q] per b, all 8 heads in ONE 2-bank psum
    [128, 8*128]: two Id.T@mask4 matmuls seed the additive causal mask
    (-30000 where k > q), 8 head matmuls KT_h.T @ QT_h accumulate on top,
    then a single ACT exp (1/8 scale fused) writes attnT bf16. QT/KT live
    in [64, H, tok] so every scores matmul reads base-partition 0 — mixing
    K=64 matmuls at PE tile rows 0 and 64 faults the device (measured, and
    walrus rejects tile_position != operand start partition).
  - outT = V''_h.T @ attnT_h: PSUM rows 0:64 get the (unnormalized) head
    output, rows 64:128 the softmax sums replicated 64x (V'' cols 64:128
    are constant 1.0) -- reciprocal + normalize are lane-aligned DVE ops.
  - y[tok,cout] = sum_ci aoT[ci].T @ Wo[ci] in bf16 (no bias matmul); y
    emission is staggered two batch elements behind so the softmax
    normalize chain never gates the PE.
"""

import math
from contextlib import ExitStack

import ml_dtypes
import numpy as np

import concourse.bass as bass
import concourse.mybir as mybir
import concourse.tile as tile
from concourse import bacc
from concourse.bass_utils import run_bass_kernel_spmd
from concourse.masks import make_identity

F32 = mybir.dt.float32
BF16 = mybir.dt.bfloat16
F8 = mybir.dt.float8e4
E4 = mybir.dt.np(mybir.dt.float8e4)  # ml_dtypes.float8_e4m3
BF = ml_dtypes.bfloat16
DRM = mybir.MatmulPerfMode.DoubleRow
AF = mybir.ActivationFunctionType

B, N, C, H = 512, 128, 512, 8
D = C // H  # 64
NCORES = 8
BPC = B // NCORES  # 64 batch elems per core
G = 4  # batch elems per group (512 tokens per projection matmul)
CT = C // 128  # 4 channel k-tiles
NG = BPC // G  # 16 groups
NVT = 8  # persistent V'' ring depth (groups g and g-1 both live)
VS = 16.0  # Wv pre-scale (residual representability)


def build_nc(bpc: int = BPC, reps: int = 1, phase_marks: list | None = None) -> bass.Bass:
    ng = bpc // G
    nc = bacc.Bacc("TRN2", target_bir_lowering=False, debug=False)

    def mark(phase):
        if phase_marks is not None:
            phase_marks.append((int(nc.get_next_instruction_name()[2:]), phase))

    xh_d = nc.dram_tensor("xh", [ng, 128, CT, G * N], F8, kind="ExternalInput").ap()
    xl_d = nc.dram_tensor("xl", [ng, 128, CT, G * N], F8, kind="ExternalInput").ap()
    wq_d = nc.dram_tensor("wq8", [128, CT, C], F8, kind="ExternalInput").ap()
    wk_d = nc.dram_tensor("wk8", [128, CT, C], F8, kind="ExternalInput").ap()
    wvh_d = nc.dram_tensor("wvh", [128, CT, C], F8, kind="ExternalInput").ap()
    wvl_d = nc.dram_tensor("wvl", [128, CT, C], F8, kind="ExternalInput").ap()
    wo_d = nc.dram_tensor("wo16", [128, CT, C], BF16, kind="ExternalInput").ap()
    bq_d = nc.dram_tensor("bq", [128, CT], F32, kind="ExternalInput").ap()
    mask_d = nc.dram_tensor("mask4", [N, 4, N], BF16, kind="ExternalInput").ap()
    y_d = nc.dram_tensor("y", [bpc, N, C], F32, kind="ExternalOutput").ap()

    with tile.TileContext(nc) as tc, ExitStack() as ctx:
        const = ctx.enter_context(tc.tile_pool(name="const", bufs=1))

        wq8 = const.tile([128, CT, C], F8, tag="wq8")
        wk8 = const.tile([128, CT, C], F8, tag="wk8")
        wvh = const.tile([128, CT, C], F8, tag="wvh")
        wvl = const.tile([128, CT, C], F8, tag="wvl")
        wo16 = const.tile([128, CT, C], BF16, tag="wo16")
        bq_sb = const.tile([128, CT], F32, tag="bq")
        mask4 = const.tile([N, 4, N], BF16, tag="mask4")
        for t, d in (
            (wq8, wq_d), (wk8, wk_d), (bq_sb, bq_d), (wvh, wvh_d),
            (wvl, wvl_d), (wo16, wo_d), (mask4, mask_d),
        ):
            nc.sync.dma_start(t[:], d)

        id128 = const.tile([128, 128], BF16, tag="id128")
        make_identity(nc, id128[:])

        # persistent V'' ring [128 tok, NVT, H, 128]; cols 64:128 of every
        # head stay 1.0 forever (memset once, on Pool — DVE is the
        # bottleneck engine; Pool is otherwise idle)
        vring = const.tile([N, NVT, H, 128], BF16, tag="vring")
        nc.gpsimd.memset(vring[:, :, :, D:], 1.0)

        # --- working pools ---
        xhp = ctx.enter_context(tc.tile_pool(name="xh", bufs=2))
        xlp = ctx.enter_context(tc.tile_pool(name="xl", bufs=2))
        qtp = ctx.enter_context(tc.tile_pool(name="qt", bufs=2))
        ktp = ctx.enter_context(tc.tile_pool(name="kt", bufs=2))
        ap_ = ctx.enter_context(tc.tile_pool(name="attnT", bufs=3))
        rp = ctx.enter_context(tc.tile_pool(name="recip", bufs=2))
        aop = ctx.enter_context(tc.tile_pool(name="aoT", bufs=4))
        yop = ctx.enter_context(tc.tile_pool(name="ysb", bufs=2))

        psA = ctx.enter_context(tc.tile_pool(name="psA", bufs=2, space="PSUM"))
        # scT (scores, dies at exp) and po (outT, dies at normalize) share a
        # 3-deep ring of 2-bank buffers: alternating alloc order means
        # po(j+1) only waits for exp(j), not for the normalize of j.
        psSO = ctx.enter_context(tc.tile_pool(name="psSO", bufs=3, space="PSUM"))

        pending_y = []  # (batch_idx, aoT tile) emitted two steps behind

        def emit_y(b, aoT):
            mark("yproj")
            yp = psA.tile([N, C], F32, tag="ps")
            for ci in range(CT):
                nc.tensor.matmul(
                    yp[:],
                    lhsT=aoT[:, ci, :],
                    rhs=wo16[:, ci, :],
                    start=(ci == 0),
                    stop=(ci == CT - 1),
                )
            y16 = yop.tile([N, C], F32, tag="ysb")
            nc.scalar.activation(y16[:], yp[:], AF.Identity)
            nc.sync.dma_start(y_d[b % bpc], y16[:])

        def emit_xload(g):
            mark("xload")
            xh = xhp.tile([128, CT, G * N], F8, tag="xh")
            xl = xlp.tile([128, CT, G * N], F8, tag="xl")
            nc.sync.dma_start(xh[:], xh_d[g])
            nc.sync.dma_start(xl[:], xl_d[g])
            return xh, xl

        def attn_a(st, j):
            # scores + exp for batch elem j of the previous group
            g, QT, KT, vslots = st
            ts = slice(j * N, (j + 1) * N)

            mark("attn")
            scT = psSO.tile([N, H, N], F32, tag="so")
            # additive causal mask seeds, one per psum bank (max moving 512)
            for half in range(2):
                nc.tensor.matmul(
                    scT[:, 4 * half : 4 * half + 4, :],
                    lhsT=id128[:],
                    rhs=mask4[:],
                    start=True,
                    stop=False,
                    skip_group_check=True,
                )
            for h in range(H):
                nc.tensor.matmul(
                    scT[:, h, :],
                    lhsT=KT[:, h, ts],
                    rhs=QT[:, h, ts],
                    start=False,
                    stop=(h % 4 == 3),
                    skip_group_check=True,
                )
            attnT = ap_.tile([N, H, N], BF16, tag="attnT")
            nc.scalar.activation(attnT[:], scT[:], AF.Exp, scale=1.0 / math.sqrt(D))
            return attnT

        def attn_b(st, j, attnT):
            # outT + normalize for batch elem j of the previous group
            g, QT, KT, vslots = st
            b = g * G + j

            mark("attn")
            # outT: rows 0:64 head output, rows 64:128 sums (x64 replicas)
            po = psSO.tile([128, H, N], F32, tag="so")
            for h in range(H):
                nc.tensor.matmul(
                    po[:, h, :],
                    lhsT=vring[:, vslots[j], h, :],
                    rhs=attnT[:, h, :],
                    start=True,
                    stop=True,
                )

            # stagger: emit an older batch elem's output projection here
            if len(pending_y) >= 2:
                emit_y(*pending_y.pop(0))

            # normalize + pack heads onto channel partitions (bf16)
            aoT = aop.tile([128, CT, N], BF16, tag="aoT")
            r64 = rp.tile([D, H, N], F32, tag="r64")
            nc.vector.reciprocal(r64[:], po[D:128, :])
            for par in range(2):
                nc.vector.tensor_mul(
                    aoT[par * D : (par + 1) * D, :, :],
                    po[0:D, par::2, :],
                    r64[:, par::2, :],
                )
            pending_y.append((b, aoT))

        # Software pipeline: group g's projections interleave with group
        # g-1's attention, split into A (scores+exp) and B (outT+normalize)
        # chunks so no engine head-of-line-blocks on the attn dependency
        # chain. B(j3) carries into the next group's emission.
        glist = [gg for _ in range(reps) for gg in range(ng)]
        x_next = emit_xload(glist[0])
        prev = None
        carry = None  # (st, j, attnT) whose B-chunk is deferred
        attns = []  # A-chunks of the current prev awaiting B

        for gi, g in enumerate(glist):
            xh, xl = x_next

            slots = []  # interleave actions, one per proj chunk

            def act_b_carry():
                nonlocal carry
                if carry is not None:
                    attn_b(*carry)
                    carry = None

            def act_a(j):
                def f():
                    attns.append((prev, j, attn_a(prev, j)))
                return f

            def act_b():
                def f():
                    st, j, attnT = attns.pop(0)
                    attn_b(st, j, attnT)
                return f

            if prev is not None:
                slots = [
                    act_b_carry,     # after Q c0: B(g-2, j3)
                    act_a(0),        # after Q c1
                    None,            # after Q c2
                    act_b(),         # after Q c3: B(j0)
                    act_a(1),        # after K c0
                    None,            # after K c1
                    act_b(),         # after K c2: B(j1)
                    act_a(2),        # after K c3
                    None,            # after V j0
                    act_b(),         # after V j1: B(j2)
                    act_a(3),        # after V j2
                    None,            # after V j3
                ]
            si = 0

            def fire():
                nonlocal si
                if si < len(slots) and slots[si] is not None:
                    slots[si]()
                si += 1

            # ---- QT / KT projections (feature-major, fp8 DoubleRow) ----
            QT = qtp.tile([D, H, G * N], BF16, tag="QT")
            KT = ktp.tile([D, H, G * N], BF16, tag="KT")
            for wname, w8, dst in (("q", wq8, QT), ("k", wk8, KT)):
                mark("qkproj")
                for co in range(CT):
                    ps = psA.tile([128, G * N], F32, tag="ps")
                    mms = [(xh, 0), (xh, 1), (xl, 0), (xl, 1)]
                    for mi, (xs, j) in enumerate(mms):
                        nc.tensor.matmul(
                            ps[:],
                            lhsT=w8[:, 2 * j : 2 * j + 2, co * 128 : (co + 1) * 128],
                            rhs=xs[:, 2 * j : 2 * j + 2, :],
                            start=(mi == 0),
                            stop=(mi == len(mms) - 1),
                            perf_mode=DRM,
                        )
                    if wname == "q":  # Q carries bq; copies on DVE (one per
                        # group rides ACT to balance the two engines)
                        nc.vector.tensor_scalar_add(
                            dst[:, 2 * co, :], ps[0:D, :], bq_sb[0:D, co : co + 1]
                        )
                        if co == CT - 1:
                            nc.scalar.activation(
                                dst[:, 2 * co + 1, :], ps[D:128, :], AF.Identity,
                                bias=bq_sb[D:128, co : co + 1],
                            )
                        else:
                            nc.vector.tensor_scalar_add(
                                dst[:, 2 * co + 1, :], ps[D:128, :],
                                bq_sb[D:128, co : co + 1],
                            )
                    else:  # K unbiased; copies on ACT
                        nc.scalar.activation(dst[:, 2 * co, :], ps[0:D, :], AF.Identity)
                        nc.scalar.activation(
                            dst[:, 2 * co + 1, :], ps[D:128, :], AF.Identity
                        )
                    fire()

            # prefetch next group's x before this group's y DMAs are queued
            if gi + 1 < len(glist):
                x_next = emit_xload(glist[gi + 1])

            # ---- V projection (token-major, 3-term scaled residual) ----
            vslots = []
            for j in range(G):
                mark("vproj")
                b = g * G + j
                ps = psA.tile([N, C], F32, tag="ps")
                mms = [(xh, wvh, 0), (xh, wvh, 1), (xh, wvl, 0), (xh, wvl, 1),
                       (xl, wvh, 0), (xl, wvh, 1)]
                for mi, (xs, ws, jj) in enumerate(mms):
                    nc.tensor.matmul(
                        ps[:],
                        lhsT=xs[:, 2 * jj : 2 * jj + 2, j * N : (j + 1) * N],
                        rhs=ws[:, 2 * jj : 2 * jj + 2, :],
                        start=(mi == 0),
                        stop=(mi == len(mms) - 1),
                        perf_mode=DRM,
                    )
                slot = b % NVT
                nc.scalar.activation(
                    vring[:, slot, :, 0:D],
                    ps.rearrange("p (h d) -> p h d", d=D),
                    AF.Identity,
                    scale=1.0 / VS,
                )
                vslots.append(slot)
                fire()

            if prev is not None:
                carry = attns.pop(0)  # A(j3) -> B deferred into next group
            prev = (g, QT, KT, vslots)

        # drain: B(last-1, j3), then full attention of the final group
        if carry is not None:
            attn_b(*carry)
            carry = None
        for j in range(G):
            at = attn_a(prev, j)
            attn_b(prev, j, at)

        while pending_y:
            emit_y(*pending_y.pop(0))

    nc.compile()
    return nc


def _e4_split(a: np.ndarray):
    hi = a.astype(E4)
    lo = (a - hi.astype(np.float32)).astype(E4)
    return hi, lo


def _w_layout(w: np.ndarray) -> np.ndarray:
    # [C, C] -> [128, CT, C] with w[p, ci, cout] = W[ci*128+p, cout]
    return np.ascontiguousarray(w.reshape(CT, 128, C).transpose(1, 0, 2))


def make_in_maps(inputs: dict) -> list[dict]:
    x = np.asarray(inputs["x"], dtype=np.float32)
    Wq, bq = np.asarray(inputs["Wq"], np.float32), np.asarray(inputs["bq"], np.float32)
    Wk = np.asarray(inputs["Wk"], np.float32)
    Wv = np.asarray(inputs["Wv"], np.float32)
    Wo = np.asarray(inputs["Wo"], np.float32)

    wq8 = _w_layout(Wq).astype(E4)
    wk8 = _w_layout(Wk).astype(E4)
    wvh, wvl = _e4_split(_w_layout(VS * Wv))
    wo16 = _w_layout(Wo).astype(BF)
    bq_l = np.ascontiguousarray(bq.reshape(CT, 128).T)  # [128, CT]

    q = np.arange(N, dtype=np.float32)
    maskT = -30000.0 * (q[None, :] < q[:, None])  # [k, q]
    mask4 = np.ascontiguousarray(
        np.broadcast_to(maskT[:, None, :], (N, 4, N))
    ).astype(BF)

    in_maps = []
    for c in range(NCORES):
        xc = x[c * BPC : (c + 1) * BPC]  # [BPC, N, C]
        xT = xc.reshape(BPC * N, C).T  # [C, T]
        xT = xT.reshape(CT, 128, NG, G * N).transpose(2, 1, 0, 3)  # [ng,128,CT,GN]
        xT = np.ascontiguousarray(xT)
        xh, xl = _e4_split(xT)
        in_maps.append(
            {
                "xh": xh,
                "xl": xl,
                "wq8": wq8,
                "wk8": wk8,
                "wvh": wvh,
                "wvl": wvl,
                "wo16": wo16,
                "bq": bq_l.astype(np.float32),
                "mask4": mask4,
            }
        )
    return in_maps


_NC_CACHE: dict[int, bass.Bass] = {}


def kernel(x, Wq, bq, Wk, bk, Wv, bv, Wo, bo, **hw_kwargs):
    inputs = dict(x=x, Wq=Wq, bq=bq, Wk=Wk, bk=bk, Wv=Wv, bv=bv, Wo=Wo, bo=bo)
    in_maps = make_in_maps(inputs)

    # attn rows sum to 1 => bv rides through Wo; both biases added on host
    bo_eff = (
        np.asarray(bv, np.float32) @ np.asarray(Wo, np.float32)
        + np.asarray(bo, np.float32)
    ).astype(np.float32)

    if BPC not in _NC_CACHE:
        _NC_CACHE[BPC] = build_nc(BPC)
    nc = _NC_CACHE[BPC]

    core_ids = list(range(NCORES))
    res = run_bass_kernel_spmd(nc, in_maps, core_ids, **hw_kwargs)
    y = np.concatenate([res.results[c]["y"] for c in core_ids], axis=0)
    y = y + bo_eff
    if hw_kwargs:
        kernel.last_result = res
    return y


# revision 11
# speedup vs baseline: 1.1161x; 1.1161x over previous
"""Fused multi-head causal self-attention block for Trainium2 (Bass/Tile).

Problem: y = MHA(x; Wq,bq,Wk,bk,Wv,bv,Wo,bo) with
  B=512, N=128 tokens, C=512 channels, H=8 heads, D=64, causal mask applied
  before the 1/sqrt(D) scaling (mask * -1e5 -> exp underflows to exactly 0).

Sharding: data-parallel over batch across 8 NeuronCores (64 batch elems per
core), weights replicated, no collectives.

Key structure (v2 — fp8 DoubleRow projections):
  - x is shipped from the host PRE-TRANSPOSED and pre-quantized to fp8-e4m3
    as a hi+lo residual pair (xT ~= xh + xl, elementwise e4m3 split), laid
    out group-contiguous [ng, 128, CT, G*N]. No on-device cast/transpose.
  - All four projections run as fp8 DoubleRow matmuls (2 K-tiles of 128 per
    instruction, 0.5 PE cycles/row):
      Q,K: psum = xh@W8 + xl@W8            (W8 = e4m3(W), plain)
      V:   psum = xh@Wh + xh@Wl + xl@Wh    (Wh/Wl = e4m3 hi/lo of 16*Wv;
           the x16 pre-scale keeps Wl out of e4m3's subnormal floor; the
           1/16 rescale rides the psum->SBUF copy for free)
    Numerics (vs fp32 reference, measured): rel-to-max ~0.0096 — the V/ao
    path is error-sensitive (linear into y) so V gets the 3-term residual;
    Q/K errors are softmax-dampened so 2-term suffices.
  - Bias algebra: softmax(s + row-const) = softmax(s) row-wise, so
    Q.bk-cross terms drop => K carries NO bias; Q keeps bq (added on the
    psum->SBUF copy). bv folds into bo (attn rows sum to 1), and bo itself
    is added on the HOST after the gather (y_dev is bias-free).
  - scores TRANSPOSED [k,q] per b, all 8 heads in ONE 2-bank psum
    [128, 8*128]: two Id.T@mask4 matmuls seed the additive causal mask
    (-30000 where k > q), 8 head matmuls KT_h.T @ QT_h accumulate on top,
    then a single ACT exp (1/8 scale fused) writes attnT bf16. QT/KT live
    in [64, H, tok] so every scores matmul reads base-partition 0 — mixing
    K=64 matmuls at PE tile rows 0 and 64 faults the device (measured, and
    walrus rejects tile_position != operand start partition).
  - outT = V''_h.T @ attnT_h: PSUM rows 0:64 get the (unnormalized) head
    output, rows 64:128 the softmax sums replicated 64x (V'' cols 64:128
    are constant 1.0) -- reciprocal + normalize are lane-aligned DVE ops.
  - y[tok,cout] = sum_ci aoT[ci].T @ Wo[ci] in bf16 (no bias matmul); y
    emission is staggered two batch elements behind so the softmax
    normalize chain never gates the PE.
"""

import math
from contextlib import ExitStack

import ml_dtypes
import numpy as np

import concourse.bass as bass
import concourse.mybir as mybir
import concourse.tile as tile
from concourse import bacc
from concourse.bass_utils import run_bass_kernel_spmd
from concourse.masks import make_identity

F32 = mybir.dt.float32
BF16 = mybir.dt.bfloat16
F8 = mybir.dt.float8e4
E4 = mybir.dt.np(mybir.dt.float8e4)  # ml_dtypes.float8_e4m3
BF = ml_dtypes.bfloat16
DRM = mybir.MatmulPerfMode.DoubleRow
AF = mybir.ActivationFunctionType

B, N, C, H = 512, 128, 512, 8
D = C // H  # 64
NCORES = 8
BPC = B // NCORES  # 64 batch elems per core
G = 4  # batch elems per group (512 tokens per projection matmul)
CT = C // 128  # 4 channel k-tiles
NG = BPC // G  # 16 groups
NVT = 8  # persistent V'' ring depth (groups g and g-1 both live)
VS = 16.0  # Wv pre-scale (residual representability)


def build_nc(bpc: int = BPC, reps: int = 1, phase_marks: list | None = None) -> bass.Bass:
    ng = bpc // G
    nc = bacc.Bacc("TRN2", target_bir_lowering=False, debug=False)

    def mark(phase):
        if phase_marks is not None:
            phase_marks.append((int(nc.get_next_instruction_name()[2:]), phase))

    xh_d = nc.dram_tensor("xh", [ng, 128, CT, G * N], F8, kind="ExternalInput").ap()
    xl_d = nc.dram_tensor("xl", [ng, 128, CT, G * N], F8, kind="ExternalInput").ap()
    wq_d = nc.dram_tensor("wq8", [128, CT, C], F8, kind="ExternalInput").ap()
    wk_d = nc.dram_tensor("wk8", [128, CT, C], F8, kind="ExternalInput").ap()
    wvh_d = nc.dram_tensor("wvh", [128, CT, C], F8, kind="ExternalInput").ap()
    wvl_d = nc.dram_tensor("wvl", [128, CT, C], F8, kind="ExternalInput").ap()
    wo_d = nc.dram_tensor("wo16", [128, CT, C], BF16, kind="ExternalInput").ap()
    bq_d = nc.dram_tensor("bq", [128, CT], F32, kind="ExternalInput").ap()
    mask_d = nc.dram_tensor("mask4", [N, 4, N], BF16, kind="ExternalInput").ap()
    y_d = nc.dram_tensor("y", [bpc, N, C], F32, kind="ExternalOutput").ap()

    with tile.TileContext(nc) as tc, ExitStack() as ctx:
        const = ctx.enter_context(tc.tile_pool(name="const", bufs=1))

        wq8 = const.tile([128, CT, C], F8, tag="wq8")
        wk8 = const.tile([128, CT, C], F8, tag="wk8")
        wvh = const.tile([128, CT, C], F8, tag="wvh")
        wvl = const.tile([128, CT, C], F8, tag="wvl")
        wo16 = const.tile([128, CT, C], BF16, tag="wo16")
        bq_sb = const.tile([128, CT], F32, tag="bq")
        mask4 = const.tile([N, 4, N], BF16, tag="mask4")
        for t, d in (
            (wq8, wq_d), (wk8, wk_d), (bq_sb, bq_d), (wvh, wvh_d),
            (wvl, wvl_d), (wo16, wo_d), (mask4, mask_d),
        ):
            nc.sync.dma_start(t[:], d)

        id128 = const.tile([128, 128], BF16, tag="id128")
        make_identity(nc, id128[:])

        # persistent V'' ring [128 tok, NVT, H, 128]; cols 64:128 of every
        # head stay 1.0 forever (memset once, on Pool — DVE is the
        # bottleneck engine; Pool is otherwise idle)
        vring = const.tile([N, NVT, H, 128], BF16, tag="vring")
        nc.gpsimd.memset(vring[:, :, :, D:], 1.0)

        # --- working pools ---
        xhp = ctx.enter_context(tc.tile_pool(name="xh", bufs=2))
        xlp = ctx.enter_context(tc.tile_pool(name="xl", bufs=2))
        qtp = ctx.enter_context(tc.tile_pool(name="qt", bufs=2))
        ktp = ctx.enter_context(tc.tile_pool(name="kt", bufs=2))
        ap_ = ctx.enter_context(tc.tile_pool(name="attnT", bufs=3))
        rp = ctx.enter_context(tc.tile_pool(name="recip", bufs=2))
        aop = ctx.enter_context(tc.tile_pool(name="aoT", bufs=4))
        yop = ctx.enter_context(tc.tile_pool(name="ysb", bufs=2))

        psA = ctx.enter_context(tc.tile_pool(name="psA", bufs=4, space="PSUM"))
        # scT (scores, dies at exp) and po (outT, dies at normalize) share a
        # ring of 2-bank buffers; with the A/B emission lag the ring never
        # head-of-line-blocks the PE.
        psSO = ctx.enter_context(tc.tile_pool(name="psSO", bufs=2, space="PSUM"))

        pending_y = []  # (batch_idx, aoT tile) emitted two steps behind

        def emit_y(b, aoT):
            mark("yproj")
            yp = psA.tile([N, C], F32, tag="ps")
            for ci in range(CT):
                nc.tensor.matmul(
                    yp[:],
                    lhsT=aoT[:, ci, :],
                    rhs=wo16[:, ci, :],
                    start=(ci == 0),
                    stop=(ci == CT - 1),
                )
            y16 = yop.tile([N, C], F32, tag="ysb")
            nc.scalar.activation(y16[:], yp[:], AF.Identity)
            nc.sync.dma_start(y_d[b % bpc], y16[:])

        def emit_xload(g):
            mark("xload")
            xh = xhp.tile([128, CT, G * N], F8, tag="xh")
            xl = xlp.tile([128, CT, G * N], F8, tag="xl")
            nc.sync.dma_start(xh[:], xh_d[g])
            nc.sync.dma_start(xl[:], xl_d[g])
            return xh, xl

        def attn_a(st, j):
            # scores + exp for batch elem j of the previous group
            g, QT, KT, vslots = st
            ts = slice(j * N, (j + 1) * N)

            mark("attn")
            scT = psSO.tile([N, H, N], F32, tag="so")
            # additive causal mask seeds, one per psum bank (max moving 512)
            for half in range(2):
                nc.tensor.matmul(
                    scT[:, 4 * half : 4 * half + 4, :],
                    lhsT=id128[:],
                    rhs=mask4[:],
                    start=True,
                    stop=False,
                    skip_group_check=True,
                )
            for h in range(H):
                nc.tensor.matmul(
                    scT[:, h, :],
                    lhsT=KT[:, h, ts],
                    rhs=QT[:, h, ts],
                    start=False,
                    stop=(h % 4 == 3),
                    skip_group_check=True,
                )
            attnT = ap_.tile([N, H, N], BF16, tag="attnT")
            nc.scalar.activation(attnT[:], scT[:], AF.Exp, scale=1.0 / math.sqrt(D))
            return attnT

        def attn_b(st, j, attnT):
            # outT + normalize for batch elem j of the previous group
            g, QT, KT, vslots = st
            b = g * G + j

            mark("attn")
            # outT: rows 0:64 head output, rows 64:128 sums (x64 replicas)
            po = psSO.tile([128, H, N], F32, tag="so")
            for h in range(H):
                nc.tensor.matmul(
                    po[:, h, :],
                    lhsT=vring[:, vslots[j], h, :],
                    rhs=attnT[:, h, :],
                    start=True,
                    stop=True,
                )

            # stagger: emit an older batch elem's output projection here
            if len(pending_y) >= 2:
                emit_y(*pending_y.pop(0))

            # normalize + pack heads onto channel partitions (bf16)
            aoT = aop.tile([128, CT, N], BF16, tag="aoT")
            r64 = rp.tile([D, H, N], F32, tag="r64")
            nc.vector.reciprocal(r64[:], po[D:128, :])
            for par in range(2):
                nc.vector.tensor_mul(
                    aoT[par * D : (par + 1) * D, :, :],
                    po[0:D, par::2, :],
                    r64[:, par::2, :],
                )
            pending_y.append((b, aoT))

        # Software pipeline: group g's projections interleave with group
        # g-1's attention, split into A (scores+exp) and B (outT+normalize)
        # chunks so no engine head-of-line-blocks on the attn dependency
        # chain. B(j3) carries into the next group's emission.
        glist = [gg for _ in range(reps) for gg in range(ng)]
        x_next = emit_xload(glist[0])
        prev = None
        carry = None  # (st, j, attnT) whose B-chunk is deferred
        attns = []  # A-chunks of the current prev awaiting B

        for gi, g in enumerate(glist):
            xh, xl = x_next

            slots = []  # interleave actions, one per proj chunk

            def act_b_carry():
                nonlocal carry
                if carry is not None:
                    attn_b(*carry)
                    carry = None

            def act_a(j):
                def f():
                    attns.append((prev, j, attn_a(prev, j)))
                return f

            def act_b():
                def f():
                    st, j, attnT = attns.pop(0)
                    attn_b(st, j, attnT)
                return f

            if prev is not None:
                slots = [
                    act_b_carry,     # after Q c0: B(g-2, j3)
                    act_a(0),        # after Q c1
                    None,            # after Q c2
                    act_b(),         # after Q c3: B(j0)
                    act_a(1),        # after K c0
                    None,            # after K c1
                    act_b(),         # after K c2: B(j1)
                    act_a(2),        # after K c3
                    None,            # after V j0
                    act_b(),         # after V j1: B(j2)
                    act_a(3),        # after V j2
                    None,            # after V j3
                ]
            si = 0

            def fire():
                nonlocal si
                if si < len(slots) and slots[si] is not None:
                    slots[si]()
                si += 1

            # ---- QT / KT projections (feature-major, fp8 DoubleRow) ----
            QT = qtp.tile([D, H, G * N], BF16, tag="QT")
            KT = ktp.tile([D, H, G * N], BF16, tag="KT")
            for wname, w8, dst in (("q", wq8, QT), ("k", wk8, KT)):
                mark("qkproj")
                for co in range(CT):
                    ps = psA.tile([128, G * N], F32, tag="ps")
                    mms = [(xh, 0), (xh, 1), (xl, 0), (xl, 1)]
                    for mi, (xs, j) in enumerate(mms):
                        nc.tensor.matmul(
                            ps[:],
                            lhsT=w8[:, 2 * j : 2 * j + 2, co * 128 : (co + 1) * 128],
                            rhs=xs[:, 2 * j : 2 * j + 2, :],
                            start=(mi == 0),
                            stop=(mi == len(mms) - 1),
                            perf_mode=DRM,
                        )
                    if wname == "q":  # Q carries bq; copies on DVE (one per
                        # group rides ACT to balance the two engines)
                        nc.vector.tensor_scalar_add(
                            dst[:, 2 * co, :], ps[0:D, :], bq_sb[0:D, co : co + 1]
                        )
                        if co == CT - 1:
                            nc.scalar.activation(
                                dst[:, 2 * co + 1, :], ps[D:128, :], AF.Identity,
                                bias=bq_sb[D:128, co : co + 1],
                            )
                        else:
                            nc.vector.tensor_scalar_add(
                                dst[:, 2 * co + 1, :], ps[D:128, :],
                                bq_sb[D:128, co : co + 1],
                            )
                    else:  # K unbiased; copies on ACT
                        nc.scalar.activation(dst[:, 2 * co, :], ps[0:D, :], AF.Identity)
                        nc.scalar.activation(
                            dst[:, 2 * co + 1, :], ps[D:128, :], AF.Identity
                        )
                    fire()

            # prefetch next group's x before this group's y DMAs are queued
            if gi + 1 < len(glist):
                x_next = emit_xload(glist[gi + 1])

            # ---- V projection (token-major, 3-term scaled residual) ----
            vslots = []
            for j in range(G):
                mark("vproj")
                b = g * G + j
                ps = psA.tile([N, C], F32, tag="ps")
                mms = [(xh, wvh, 0), (xh, wvh, 1), (xh, wvl, 0), (xh, wvl, 1),
                       (xl, wvh, 0), (xl, wvh, 1)]
                for mi, (xs, ws, jj) in enumerate(mms):
                    nc.tensor.matmul(
                        ps[:],
                        lhsT=xs[:, 2 * jj : 2 * jj + 2, j * N : (j + 1) * N],
                        rhs=ws[:, 2 * jj : 2 * jj + 2, :],
                        start=(mi == 0),
                        stop=(mi == len(mms) - 1),
                        perf_mode=DRM,
                    )
                slot = b % NVT
                nc.scalar.activation(
                    vring[:, slot, :, 0:D],
                    ps.rearrange("p (h d) -> p h d", d=D),
                    AF.Identity,
                    scale=1.0 / VS,
                )
                vslots.append(slot)
                fire()

            if prev is not None:
                carry = attns.pop(0)  # A(j3) -> B deferred into next group
            prev = (g, QT, KT, vslots)

        # drain: B(last-1, j3), then full attention of the final group
        if carry is not None:
            attn_b(*carry)
            carry = None
        for j in range(G):
            at = attn_a(prev, j)
            attn_b(prev, j, at)

        while pending_y:
            emit_y(*pending_y.pop(0))

    nc.compile()
    return nc


def _e4_split(a: np.ndarray):
    hi = a.astype(E4)
    lo = (a - hi.astype(np.float32)).astype(E4)
    return hi, lo


def _w_layout(w: np.ndarray) -> np.ndarray:
    # [C, C] -> [128, CT, C] with w[p, ci, cout] = W[ci*128+p, cout]
    return np.ascontiguousarray(w.reshape(CT, 128, C).transpose(1, 0, 2))


def make_in_maps(inputs: dict) -> list[dict]:
    x = np.asarray(inputs["x"], dtype=np.float32)
    Wq, bq = np.asarray(inputs["Wq"], np.float32), np.asarray(inputs["bq"], np.float32)
    Wk = np.asarray(inputs["Wk"], np.float32)
    Wv = np.asarray(inputs["Wv"], np.float32)
    Wo = np.asarray(inputs["Wo"], np.float32)

    wq8 = _w_layout(Wq).astype(E4)
    wk8 = _w_layout(Wk).astype(E4)
    wvh, wvl = _e4_split(_w_layout(VS * Wv))
    wo16 = _w_layout(Wo).astype(BF)
    bq_l = np.ascontiguousarray(bq.reshape(CT, 128).T)  # [128, CT]

    q = np.arange(N, dtype=np.float32)
    maskT = -30000.0 * (q[None, :] < q[:, None])  # [k, q]
    mask4 = np.ascontiguousarray(
        np.broadcast_to(maskT[:, None, :], (N, 4, N))
    ).astype(BF)

    in_maps = []
    for c in range(NCORES):
        xc = x[c * BPC : (c + 1) * BPC]  # [BPC, N, C]
        xT = xc.reshape(BPC * N, C).T  # [C, T]
        xT = xT.reshape(CT, 128, NG, G * N).transpose(2, 1, 0, 3)  # [ng,128,CT,GN]
        xT = np.ascontiguousarray(xT)
        xh, xl = _e4_split(xT)
        in_maps.append(
            {
                "xh": xh,
                "xl": xl,
                "wq8": wq8,
                "wk8": wk8,
                "wvh": wvh,
                "wvl": wvl,
                "wo16": wo16,
                "bq": bq_l.astype(np.float32),
                "mask4": mask4,
            }
        )
    return in_maps


_NC_CACHE: dict[int, bass.Bass] = {}


def kernel(x, Wq, bq, Wk, bk, Wv, bv, Wo, bo, **hw_kwargs):
    inputs = dict(x=x, Wq=Wq, bq=bq, Wk=Wk, bk=bk, Wv=Wv, bv=bv, Wo=Wo, bo=bo)
    in_maps = make_in_maps(inputs)

    # attn rows sum to 1 => bv rides through Wo; both biases added on host
    bo_eff = (
        np.asarray(bv, np.float32) @ np.asarray(Wo, np.float32)
        + np.asarray(bo, np.float32)
    ).astype(np.float32)

    if BPC not in _NC_CACHE:
        _NC_CACHE[BPC] = build_nc(BPC)
    nc = _NC_CACHE[BPC]

    core_ids = list(range(NCORES))
    res = run_bass_kernel_spmd(nc, in_maps, core_ids, **hw_kwargs)
    y = np.concatenate([res.results[c]["y"] for c in core_ids], axis=0)
    y = y + bo_eff
    if hw_kwargs:
        kernel.last_result = res
    return y


# revision 13
# speedup vs baseline: 1.1532x; 1.0332x over previous
"""Fused multi-head causal self-attention block for Trainium2 (Bass/Tile).

Problem: y = MHA(x; Wq,bq,Wk,bk,Wv,bv,Wo,bo) with
  B=512, N=128 tokens, C=512 channels, H=8 heads, D=64, causal mask applied
  before the 1/sqrt(D) scaling (mask * -1e5 -> exp underflows to exactly 0).

Sharding: data-parallel over batch across 8 NeuronCores (64 batch elems per
core), weights replicated, no collectives.

Key structure (v2 — fp8 DoubleRow projections):
  - x is shipped from the host PRE-TRANSPOSED and pre-quantized to fp8-e4m3
    as a hi+lo residual pair (xT ~= xh + xl, elementwise e4m3 split), laid
    out group-contiguous [ng, 128, CT, G*N]. No on-device cast/transpose.
  - All four projections run as fp8 DoubleRow matmuls (2 K-tiles of 128 per
    instruction, 0.5 PE cycles/row):
      Q,K: psum = xh@W8 + xl@W8            (W8 = e4m3(W), plain)
      V:   psum = xh@Wh + xh@Wl + xl@Wh    (Wh/Wl = e4m3 hi/lo of 16*Wv;
           the x16 pre-scale keeps Wl out of e4m3's subnormal floor; the
           1/16 rescale rides the psum->SBUF copy for free)
    Numerics (vs fp32 reference, measured): rel-to-max ~0.0096 — the V/ao
    path is error-sensitive (linear into y) so V gets the 3-term residual;
    Q/K errors are softmax-dampened so 2-term suffices.
  - Bias algebra: softmax(s + row-const) = softmax(s) row-wise, so
    Q.bk-cross terms drop => K carries NO bias; Q keeps bq (added on the
    psum->SBUF copy). bv folds into bo (attn rows sum to 1), and bo itself
    is added on the HOST after the gather (y_dev is bias-free).
  - scores TRANSPOSED [k,q] per b, all 8 heads in ONE 2-bank psum
    [128, 8*128]: two Id.T@mask4 matmuls seed the additive causal mask
    (-30000 where k > q), 8 head matmuls KT_h.T @ QT_h accumulate on top,
    then a single ACT exp (1/8 scale fused) writes attnT bf16. QT/KT live
    in [64, H, tok] so every scores matmul reads base-partition 0 — mixing
    K=64 matmuls at PE tile rows 0 and 64 faults the device (measured, and
    walrus rejects tile_position != operand start partition).
  - outT = V''_h.T @ attnT_h: PSUM rows 0:64 get the (unnormalized) head
    output, rows 64:128 the softmax sums replicated 64x (V'' cols 64:128
    are constant 1.0) -- reciprocal + normalize are lane-aligned DVE ops.
  - y[tok,cout] = sum_ci aoT[ci].T @ Wo[ci] in bf16 (no bias matmul); y
    emission is staggered two batch elements behind so the softmax
    normalize chain never gates the PE.
"""

import math
from contextlib import ExitStack

import ml_dtypes
import numpy as np

import concourse.bass as bass
import concourse.mybir as mybir
import concourse.tile as tile
from concourse import bacc
from concourse.bass_utils import run_bass_kernel_spmd
from concourse.masks import make_identity

F32 = mybir.dt.float32
BF16 = mybir.dt.bfloat16
F8 = mybir.dt.float8e4
E4 = mybir.dt.np(mybir.dt.float8e4)  # ml_dtypes.float8_e4m3
BF = ml_dtypes.bfloat16
DRM = mybir.MatmulPerfMode.DoubleRow
AF = mybir.ActivationFunctionType

B, N, C, H = 512, 128, 512, 8
D = C // H  # 64
NCORES = 8
BPC = B // NCORES  # 64 batch elems per core
G = 4  # batch elems per group (512 tokens per projection matmul)
CT = C // 128  # 4 channel k-tiles
NG = BPC // G  # 16 groups
NVT = 8  # persistent V'' ring depth (groups g and g-1 both live)
VS = 16.0  # Wv pre-scale (residual representability)


def build_nc(bpc: int = BPC, reps: int = 1, phase_marks: list | None = None) -> bass.Bass:
    ng = bpc // G
    nc = bacc.Bacc("TRN2", target_bir_lowering=False, debug=False)

    def mark(phase):
        if phase_marks is not None:
            phase_marks.append((int(nc.get_next_instruction_name()[2:]), phase))

    xh_d = nc.dram_tensor("xh", [ng, 128, CT, G * N], F8, kind="ExternalInput").ap()
    xl_d = nc.dram_tensor("xl", [ng, 128, CT, G * N], F8, kind="ExternalInput").ap()
    wq_d = nc.dram_tensor("wq8", [128, CT, C], F8, kind="ExternalInput").ap()
    wk_d = nc.dram_tensor("wk8", [128, CT, C], F8, kind="ExternalInput").ap()
    wvh_d = nc.dram_tensor("wvh", [128, CT, C], F8, kind="ExternalInput").ap()
    wvl_d = nc.dram_tensor("wvl", [128, CT, C], F8, kind="ExternalInput").ap()
    wo_d = nc.dram_tensor("wo16", [128, CT, C], BF16, kind="ExternalInput").ap()
    bq_d = nc.dram_tensor("bq", [128, CT], F32, kind="ExternalInput").ap()
    mask_d = nc.dram_tensor("mask4", [N, 4, N], BF16, kind="ExternalInput").ap()
    y_d = nc.dram_tensor("y", [bpc, N, C], F32, kind="ExternalOutput").ap()

    with tile.TileContext(nc) as tc, ExitStack() as ctx:
        const = ctx.enter_context(tc.tile_pool(name="const", bufs=1))

        wq8 = const.tile([128, CT, C], F8, tag="wq8")
        wk8 = const.tile([128, CT, C], F8, tag="wk8")
        wvh = const.tile([128, CT, C], F8, tag="wvh")
        wvl = const.tile([128, CT, C], F8, tag="wvl")
        wo16 = const.tile([128, CT, C], BF16, tag="wo16")
        bq_sb = const.tile([128, CT], F32, tag="bq")
        mask4 = const.tile([N, 4, N], BF16, tag="mask4")
        for t, d in (
            (wq8, wq_d), (wk8, wk_d), (bq_sb, bq_d), (wvh, wvh_d),
            (wvl, wvl_d), (wo16, wo_d), (mask4, mask_d),
        ):
            nc.sync.dma_start(t[:], d)

        id128 = const.tile([128, 128], BF16, tag="id128")
        make_identity(nc, id128[:])

        # persistent V'' ring [128 tok, NVT, H, 128]; cols 64:128 of every
        # head stay 1.0 forever (memset once, on Pool — DVE is the
        # bottleneck engine; Pool is otherwise idle)
        vring = const.tile([N, NVT, H, 128], BF16, tag="vring")
        nc.gpsimd.memset(vring[:, :, :, D:], 1.0)

        # --- working pools ---
        xhp = ctx.enter_context(tc.tile_pool(name="xh", bufs=2))
        xlp = ctx.enter_context(tc.tile_pool(name="xl", bufs=2))
        qtp = ctx.enter_context(tc.tile_pool(name="qt", bufs=2))
        ktp = ctx.enter_context(tc.tile_pool(name="kt", bufs=2))
        ap_ = ctx.enter_context(tc.tile_pool(name="attnT", bufs=3))
        rp = ctx.enter_context(tc.tile_pool(name="recip", bufs=2))
        aop = ctx.enter_context(tc.tile_pool(name="aoT", bufs=4))
        yop = ctx.enter_context(tc.tile_pool(name="ysb", bufs=2))

        psA = ctx.enter_context(tc.tile_pool(name="psA", bufs=4, space="PSUM"))
        # scT (scores, dies at exp) and po (outT, dies at normalize) share a
        # ring of 2-bank buffers; with the A/B emission lag the ring never
        # head-of-line-blocks the PE.
        psSO = ctx.enter_context(tc.tile_pool(name="psSO", bufs=2, space="PSUM"))

        pending_y = []  # (batch_idx, aoT tile) emitted two steps behind

        def emit_y(b, aoT):
            mark("yproj")
            yp = psA.tile([N, C], F32, tag="ps")
            for ci in range(CT):
                nc.tensor.matmul(
                    yp[:],
                    lhsT=aoT[:, ci, :],
                    rhs=wo16[:, ci, :],
                    start=(ci == 0),
                    stop=(ci == CT - 1),
                )
            y16 = yop.tile([N, C], F32, tag="ysb")
            nc.scalar.activation(y16[:], yp[:], AF.Identity)
            nc.sync.dma_start(y_d[b % bpc], y16[:])

        def emit_xload(g):
            mark("xload")
            xh = xhp.tile([128, CT, G * N], F8, tag="xh")
            xl = xlp.tile([128, CT, G * N], F8, tag="xl")
            nc.sync.dma_start(xh[:], xh_d[g])
            nc.sync.dma_start(xl[:], xl_d[g])
            return xh, xl

        def attn_a(st, j):
            # scores + exp for batch elem j of the previous group
            g, QT, KT, vslots = st
            ts = slice(j * N, (j + 1) * N)

            mark("attn")
            scT = psSO.tile([N, H, N], F32, tag="so")
            # additive causal mask seeds, one per psum bank (max moving 512)
            for half in range(2):
                nc.tensor.matmul(
                    scT[:, 4 * half : 4 * half + 4, :],
                    lhsT=id128[:],
                    rhs=mask4[:],
                    start=True,
                    stop=False,
                    skip_group_check=True,
                )
            for h in range(H):
                nc.tensor.matmul(
                    scT[:, h, :],
                    lhsT=KT[:, h, ts],
                    rhs=QT[:, h, ts],
                    start=False,
                    stop=(h % 4 == 3),
                    skip_group_check=True,
                )
            attnT = ap_.tile([N, H, N], BF16, tag="attnT")
            nc.scalar.activation(attnT[:], scT[:], AF.Exp, scale=1.0 / math.sqrt(D))
            return attnT

        def attn_b(st, j, attnT):
            # outT + normalize for batch elem j of the previous group
            g, QT, KT, vslots = st
            b = g * G + j

            mark("attn")
            # outT: rows 0:64 head output, rows 64:128 sums (x64 replicas)
            po = psSO.tile([128, H, N], F32, tag="so")
            for h in range(H):
                nc.tensor.matmul(
                    po[:, h, :],
                    lhsT=vring[:, vslots[j], h, :],
                    rhs=attnT[:, h, :],
                    start=True,
                    stop=True,
                )

            # stagger: emit an older batch elem's output projection here
            if len(pending_y) >= 2:
                emit_y(*pending_y.pop(0))

            # normalize + pack heads onto channel partitions (bf16)
            aoT = aop.tile([128, CT, N], BF16, tag="aoT")
            r64 = rp.tile([D, H, N], F32, tag="r64")
            nc.vector.reciprocal(r64[:], po[D:128, :])
            for par in range(2):
                nc.vector.tensor_mul(
                    aoT[par * D : (par + 1) * D, :, :],
                    po[0:D, par::2, :],
                    r64[:, par::2, :],
                )
            pending_y.append((b, aoT))

        # Software pipeline: group g's projections interleave with group
        # g-1's attention, split into A (scores+exp) and B (outT+normalize)
        # chunks so no engine head-of-line-blocks on the attn dependency
        # chain. B(j3) carries into the next group's emission.
        glist = [gg for _ in range(reps) for gg in range(ng)]
        x_next = emit_xload(glist[0])
        prev = None
        carry = None  # (st, j, attnT) whose B-chunk is deferred
        attns = []  # A-chunks of the current prev awaiting B

        for gi, g in enumerate(glist):
            xh, xl = x_next

            slots = []  # interleave actions, one per proj chunk

            def act_b_carry():
                nonlocal carry
                if carry is not None:
                    attn_b(*carry)
                    carry = None

            def act_a(j):
                def f():
                    attns.append((prev, j, attn_a(prev, j)))
                return f

            def act_b():
                def f():
                    st, j, attnT = attns.pop(0)
                    attn_b(st, j, attnT)
                return f

            if prev is not None:
                slots = [
                    act_a(0),        # after Q c0
                    None,            # after Q c1
                    act_b(),         # after Q c2: B(j0)
                    act_a(1),        # after Q c3
                    None,            # after K c0
                    act_b(),         # after K c1: B(j1)
                    act_a(2),        # after K c2
                    None,            # after K c3
                    act_b(),         # after V j0: B(j2)
                    act_a(3),        # after V j1
                    None,            # after V j2
                    act_b(),         # after V j3: B(j3)
                ]
            si = 0

            def fire():
                nonlocal si
                if si < len(slots) and slots[si] is not None:
                    slots[si]()
                si += 1

            # ---- QT / KT projections (feature-major, fp8 DoubleRow) ----
            QT = qtp.tile([D, H, G * N], BF16, tag="QT")
            KT = ktp.tile([D, H, G * N], BF16, tag="KT")
            for wname, w8, dst in (("q", wq8, QT), ("k", wk8, KT)):
                mark("qkproj")
                for co in range(CT):
                    ps = psA.tile([128, G * N], F32, tag="ps")
                    mms = [(xh, 0), (xh, 1), (xl, 0), (xl, 1)]
                    for mi, (xs, j) in enumerate(mms):
                        nc.tensor.matmul(
                            ps[:],
                            lhsT=w8[:, 2 * j : 2 * j + 2, co * 128 : (co + 1) * 128],
                            rhs=xs[:, 2 * j : 2 * j + 2, :],
                            start=(mi == 0),
                            stop=(mi == len(mms) - 1),
                            perf_mode=DRM,
                        )
                    if wname == "q":  # Q carries bq; copies on DVE (one per
                        # group rides ACT to balance the two engines)
                        nc.vector.tensor_scalar_add(
                            dst[:, 2 * co, :], ps[0:D, :], bq_sb[0:D, co : co + 1]
                        )
                        if co == CT - 1:
                            nc.scalar.activation(
                                dst[:, 2 * co + 1, :], ps[D:128, :], AF.Identity,
                                bias=bq_sb[D:128, co : co + 1],
                            )
                        else:
                            nc.vector.tensor_scalar_add(
                                dst[:, 2 * co + 1, :], ps[D:128, :],
                                bq_sb[D:128, co : co + 1],
                            )
                    else:  # K unbiased; copies on ACT
                        nc.scalar.activation(dst[:, 2 * co, :], ps[0:D, :], AF.Identity)
                        nc.scalar.activation(
                            dst[:, 2 * co + 1, :], ps[D:128, :], AF.Identity
                        )
                    fire()

            # prefetch next group's x before this group's y DMAs are queued
            if gi + 1 < len(glist):
                x_next = emit_xload(glist[gi + 1])

            # ---- V projection (token-major, 3-term scaled residual) ----
            vslots = []
            for j in range(G):
                mark("vproj")
                b = g * G + j
                ps = psA.tile([N, C], F32, tag="ps")
                mms = [(xh, wvh, 0), (xh, wvh, 1), (xh, wvl, 0), (xh, wvl, 1),
                       (xl, wvh, 0), (xl, wvh, 1)]
                for mi, (xs, ws, jj) in enumerate(mms):
                    nc.tensor.matmul(
                        ps[:],
                        lhsT=xs[:, 2 * jj : 2 * jj + 2, j * N : (j + 1) * N],
                        rhs=ws[:, 2 * jj : 2 * jj + 2, :],
                        start=(mi == 0),
                        stop=(mi == len(mms) - 1),
                        perf_mode=DRM,
                    )
                slot = b % NVT
                nc.scalar.activation(
                    vring[:, slot, :, 0:D],
                    ps.rearrange("p (h d) -> p h d", d=D),
                    AF.Identity,
                    scale=1.0 / VS,
                )
                vslots.append(slot)
                fire()

            prev = (g, QT, KT, vslots)

        # drain: full attention of the final group
        for j in range(G):
            at = attn_a(prev, j)
            attn_b(prev, j, at)

        while pending_y:
            emit_y(*pending_y.pop(0))

    nc.compile()
    return nc


def _e4_split(a: np.ndarray):
    hi = a.astype(E4)
    lo = (a - hi.astype(np.float32)).astype(E4)
    return hi, lo


def _w_layout(w: np.ndarray) -> np.ndarray:
    # [C, C] -> [128, CT, C] with w[p, ci, cout] = W[ci*128+p, cout]
    return np.ascontiguousarray(w.reshape(CT, 128, C).transpose(1, 0, 2))


def make_in_maps(inputs: dict) -> list[dict]:
    x = np.asarray(inputs["x"], dtype=np.float32)
    Wq, bq = np.asarray(inputs["Wq"], np.float32), np.asarray(inputs["bq"], np.float32)
    Wk = np.asarray(inputs["Wk"], np.float32)
    Wv = np.asarray(inputs["Wv"], np.float32)
    Wo = np.asarray(inputs["Wo"], np.float32)

    wq8 = _w_layout(Wq).astype(E4)
    wk8 = _w_layout(Wk).astype(E4)
    wvh, wvl = _e4_split(_w_layout(VS * Wv))
    wo16 = _w_layout(Wo).astype(BF)
    bq_l = np.ascontiguousarray(bq.reshape(CT, 128).T)  # [128, CT]

    q = np.arange(N, dtype=np.float32)
    maskT = -30000.0 * (q[None, :] < q[:, None])  # [k, q]
    mask4 = np.ascontiguousarray(
        np.broadcast_to(maskT[:, None, :], (N, 4, N))
    ).astype(BF)

    in_maps = []
    for c in range(NCORES):
        xc = x[c * BPC : (c + 1) * BPC]  # [BPC, N, C]
        xT = xc.reshape(BPC * N, C).T  # [C, T]
        xT = xT.reshape(CT, 128, NG, G * N).transpose(2, 1, 0, 3)  # [ng,128,CT,GN]
        xT = np.ascontiguousarray(xT)
        xh, xl = _e4_split(xT)
        in_maps.append(
            {
                "xh": xh,
                "xl": xl,
                "wq8": wq8,
                "wk8": wk8,
                "wvh": wvh,
                "wvl": wvl,
                "wo16": wo16,
                "bq": bq_l.astype(np.float32),
                "mask4": mask4,
            }
        )
    return in_maps


_NC_CACHE: dict[int, bass.Bass] = {}


def kernel(x, Wq, bq, Wk, bk, Wv, bv, Wo, bo, **hw_kwargs):
    inputs = dict(x=x, Wq=Wq, bq=bq, Wk=Wk, bk=bk, Wv=Wv, bv=bv, Wo=Wo, bo=bo)
    in_maps = make_in_maps(inputs)

    # attn rows sum to 1 => bv rides through Wo; both biases added on host
    bo_eff = (
        np.asarray(bv, np.float32) @ np.asarray(Wo, np.float32)
        + np.asarray(bo, np.float32)
    ).astype(np.float32)

    if BPC not in _NC_CACHE:
        _NC_CACHE[BPC] = build_nc(BPC)
    nc = _NC_CACHE[BPC]

    core_ids = list(range(NCORES))
    res = run_bass_kernel_spmd(nc, in_maps, core_ids, **hw_kwargs)
    y = np.concatenate([res.results[c]["y"] for c in core_ids], axis=0)
    y = y + bo_eff
    if hw_kwargs:
        kernel.last_result = res
    return y
